# revision 12
# baseline (speedup 1.0000x reference)
"""GAT (2-layer graph attention) Trainium2 kernel, 8-core SPMD.

Sharding: rows of N are sharded across 8 cores (512 rows each); every core
computes the full first-layer projections h (replicated) but only its own
rows' attention. Between layers the per-core x_cat rows are gathered on the
host and layer 2 runs as a second launch.

Math per layer (per batch b, head o):
    h = x @ W^T, f1 = x (W^T a1), f2 = x (W^T a2)
    P^T[j,i] = exp(lrelu(f1_i + f2_j + M[j,i]))   M = (adj^T-1)*3e4 (mask)
    [U | Z] = P^T.T @ [h | 1]  (PE accumulates over j-chunks)
    out_i = U_i / Z_i  (+ ELU for layer 1)

Layout: scores kept transposed [j(part), i(free)] so the PE contraction dim
(j) is on partitions for both operands; Z comes free via the ones column.
f2 is produced partition-major by tiny N=PAIR matmuls that reuse the already
loaded x^T weights; f1 (free-major) by M=1 matmuls + a ones outer-product.
"""

import os
from contextlib import ExitStack

import numpy as np
import ml_dtypes

import concourse.bacc as bacc
import concourse.tile as tile
import concourse.mybir as mybir
from concourse.bass_utils import run_bass_kernel_spmd

BF16 = mybir.dt.bfloat16
F32 = mybir.dt.float32
NPBF16 = ml_dtypes.bfloat16
AFT = mybir.ActivationFunctionType
ALU = mybir.AluOpType

NCORES = 8
B, N, F_IN, H, HID, OUT = 2, 4096, 512, 4, 128, 64
RPC = N // NCORES          # 512 rows per core
NJC = N // 128             # 32 column (j) chunks
NKC = F_IN // 128          # 4 contraction chunks for projections
NIS = RPC // 128           # 4 row (i) subtiles per core
MASKVAL = 30000.0
JGRP = 4                   # j-chunks per batched exp group
NGRP = NJC // JGRP
MSG = 8                    # ms per xT slab load

# engine-assignment config (tuned against TimelineSim)
CFG = {
    "tt_dve_mod": 4,        # every k-th u-build TT stays on DVE (rest gpsimd)
}

_cache = {}

last_exec_ns = []


def _build_layer(nheads, hid, label):
    nc = bacc.Bacc("TRN2", target_bir_lowering=False, debug=False,
                   enable_asserts=True, num_devices=NCORES)

    PAIR = 2 if nheads % 2 == 0 else 1
    NPR = nheads // PAIR   # head pairs
    HS = hid + 4           # hsb per-head block stride: [h | 1 | pad]
    RH = hid + 1           # attn rhs width: [h | 1]

    xT = nc.dram_tensor("xT", [B, F_IN, N], BF16, kind="ExternalInput")
    xT_own = nc.dram_tensor("xT_own", [B, F_IN, RPC], BF16, kind="ExternalInput")
    maskT = nc.dram_tensor("maskT", [B, N, RPC], BF16, kind="ExternalInput")
    w_aug = nc.dram_tensor("w_aug", [nheads, NKC, 128, hid], BF16,
                           kind="ExternalInput")
    w1 = nc.dram_tensor("w1", [nheads, NKC, 128, 1], BF16, kind="ExternalInput")
    w2 = nc.dram_tensor("w2", [nheads, NKC, 128, 1], BF16, kind="ExternalInput")
    if label == "l1":
        out_d = nc.dram_tensor("xcat", [B, RPC, nheads * hid], BF16,
                               kind="ExternalOutput")
    else:
        out_d = nc.dram_tensor("out", [B, RPC, hid], F32, kind="ExternalOutput")

    with tile.TileContext(nc) as tc, ExitStack() as ctx:
        const_pool = ctx.enter_context(tc.tile_pool(name="const", bufs=1))
        mask_pool = ctx.enter_context(tc.tile_pool(name="mask", bufs=2))
        xt_pool = ctx.enter_context(tc.tile_pool(name="xt", bufs=2 * NKC))
        xto_pool = ctx.enter_context(tc.tile_pool(name="xto", bufs=2 * NKC))
        hsb_pool = ctx.enter_context(tc.tile_pool(name="hsb", bufs=NPR + 1))
        f2s_pool = ctx.enter_context(tc.tile_pool(name="f2s", bufs=NPR + 1))
        f1b_pool = ctx.enter_context(tc.tile_pool(name="f1b", bufs=nheads + 1))
        f1r_pool = ctx.enter_context(tc.tile_pool(name="f1r", bufs=2))
        u_pool = ctx.enter_context(tc.tile_pool(name="u", bufs=3))
        w_pool = ctx.enter_context(tc.tile_pool(name="wb", bufs=2))
        p_pool = ctx.enter_context(tc.tile_pool(name="pb", bufs=2))
        eps_pool = ctx.enter_context(tc.tile_pool(name="eps", bufs=2))
        out_pool = ctx.enter_context(tc.tile_pool(name="outp", bufs=2 * NIS))
        hpsum = ctx.enter_context(tc.tile_pool(name="hpsum", bufs=4, space="PSUM"))
        apsum = ctx.enter_context(tc.tile_pool(name="apsum", bufs=4, space="PSUM"))

        # constants: waug_sb holds [W^T_a | W^T_b] per (pair, kc); w2_sb holds
        # [w2_a | w2_b] per (pair, kc); w1_sb one col per (head, kc)
        waug_sb = const_pool.tile([128, NPR * NKC * PAIR * hid], BF16)
        w1_sb = const_pool.tile([128, nheads * NKC], BF16)
        w2_sb = const_pool.tile([128, NPR * NKC * PAIR], BF16)
        for o in range(nheads):
            pr, q = o // PAIR, o % PAIR
            for kc in range(NKC):
                i = (pr * NKC + kc) * PAIR + q
                nc.sync.dma_start(waug_sb[:, i * hid:(i + 1) * hid],
                                  w_aug.ap()[o, kc])
                nc.sync.dma_start(w1_sb[:, (o * NKC + kc):(o * NKC + kc) + 1],
                                  w1.ap()[o, kc])
                nc.sync.dma_start(w2_sb[:, i:i + 1], w2.ap()[o, kc])
        ones_sb = const_pool.tile([1, 128], BF16)
        nc.vector.memset(ones_sb[:], 1.0)

        for b in range(B):
            masksb = mask_pool.tile([128, NJC * RPC], BF16, tag="masksb")
            nc.sync.dma_start(
                masksb[:].rearrange("p (c i) -> p c i", i=RPC),
                maskT.ap()[b].rearrange("(c p) i -> p c i", p=128))

            # --- f1 (own rows, free-major): M=1 matmuls + ones outer-product
            xto = []
            for kc in range(NKC):
                t = xto_pool.tile([128, RPC], BF16, tag="xto")
                nc.sync.dma_start(t[:], xT_own.ap()[b, kc * 128:(kc + 1) * 128, :])
                xto.append(t)
            f1bs = []
            for o in range(nheads):
                fps = hpsum.tile([128, 512], F32, tag="hps", name="fps")
                for kc in range(NKC):
                    nc.tensor.matmul(fps[0:1, :RPC],
                                     w1_sb[:, o * NKC + kc:o * NKC + kc + 1],
                                     xto[kc][:, :],
                                     start=(kc == 0), stop=(kc == NKC - 1))
                f1r = f1r_pool.tile([1, RPC], BF16, tag="f1r")
                nc.vector.tensor_copy(f1r[:], fps[0:1, :RPC])
                bps = hpsum.tile([128, 512], F32, tag="hps")
                nc.tensor.matmul(bps[:, :RPC], ones_sb[:, :], f1r[:, :],
                                 start=True, stop=True)
                f1b = f1b_pool.tile([128, RPC], BF16, tag="f1b")
                nc.scalar.copy(f1b[:], bps[:, :RPC])
                f1bs.append(f1b)

            # --- projections: h (pairs) + f2 (partition-major) ---
            hsbs = []    # per pair: [128, NJC*PAIR*HS]
            for pr in range(NPR):
                hs = hsb_pool.tile([128, NJC * PAIR * HS], BF16, tag="hs")
                hv = hs[:].rearrange("p (c q f) -> p c q f", q=PAIR, f=HS)
                nc.vector.memset(hv[:, :, :, hid:hid + 1], 1.0)
                hsbs.append(hs)
            f2ps = [hpsum.tile([128, 512], F32, tag="hps", name="f2ps")
                    for _ in range(NPR)]
            f2sbs = [f2s_pool.tile([128, NJC * PAIR], F32, tag="f2sb",
                                   name="f2sb")
                     for _ in range(NPR)]
            for ms in range(NJC):
                if ms % MSG == 0:
                    xslabs = []
                    for kc in range(NKC):
                        xs = xt_pool.tile([128, MSG * 128], BF16, tag="xt")
                        nc.sync.dma_start(
                            xs[:],
                            xT.ap()[b, kc * 128:(kc + 1) * 128,
                                    ms * 128:(ms + MSG) * 128])
                        xslabs.append(xs)
                hps = [hpsum.tile([128, 512], F32, tag="hps", name="hps")
                       for _ in range(NPR)]
                for kc in range(NKC):
                    xt_t = xslabs[kc][:, (ms % MSG) * 128:(ms % MSG + 1) * 128]
                    st = (kc == 0)
                    sp = (kc == NKC - 1)
                    for pr in range(NPR):
                        i = (pr * NKC + kc) * PAIR
                        nc.tensor.matmul(
                            hps[pr][:, :PAIR * hid], xt_t,
                            waug_sb[:, i * hid:(i + PAIR) * hid],
                            start=st, stop=sp)
                        nc.tensor.matmul(
                            f2ps[pr][:, ms * PAIR:(ms + 1) * PAIR], xt_t,
                            w2_sb[:, i:i + PAIR],
                            start=st, stop=sp)
                for pr in range(NPR):
                    hv = hsbs[pr][:].rearrange("p (c q f) -> p c q f",
                                               q=PAIR, f=HS)
                    pv = hps[pr][:, :PAIR * hid].rearrange(
                        "p (q f) -> p q f", f=hid)
                    nc.scalar.copy(hv[:, ms, :, 0:hid], pv)
                # f2 to fp32 SBUF incrementally so attention can pipeline
                if ms % JGRP == JGRP - 1:
                    lo = (ms - (JGRP - 1)) * PAIR
                    hi = (ms + 1) * PAIR
                    for pr in range(NPR):
                        nc.vector.tensor_copy(f2sbs[pr][:, lo:hi],
                                              f2ps[pr][:, lo:hi])

            # --- attention per head ---
            if label == "l1":
                xcts = [out_pool.tile([128, nheads * hid], BF16, tag="xct",
                                      name="xct")
                        for _ in range(NIS)]
            for o in range(nheads):
                pr, q = o // PAIR, o % PAIR
                hs = hsbs[pr]
                f2sb = f2sbs[pr]

                def blk(jc):
                    return (jc * PAIR + q) * HS

                aps = [apsum.tile([128, 512], F32, tag="aps", name="aps")
                       for _ in range(NIS)]
                for g in range(NGRP):
                    wb = w_pool.tile([128, JGRP * RPC], BF16, tag="wb")
                    ub = u_pool.tile([128, JGRP * RPC], BF16, tag="ub")
                    for j8 in range(JGRP):
                        jc = g * JGRP + j8
                        usl = ub[:, j8 * RPC:(j8 + 1) * RPC]
                        gidx = ((b * nheads + o) * NGRP + g) * JGRP + j8
                        m = CFG["tt_dve_mod"]
                        tte = nc.vector if (m and gidx % m == 0) else nc.gpsimd
                        tte.tensor_add(usl, masksb[:, jc * RPC:(jc + 1) * RPC],
                                       f1bs[o][:])
                        nc.vector.tensor_scalar(
                            usl, usl, f2sb[:, jc * PAIR + q:jc * PAIR + q + 1],
                            None, ALU.add)
                    # lrelu: one batched STT on DVE per group
                    nc.vector.scalar_tensor_tensor(
                        wb[:], ub[:], 0.2, ub[:], ALU.mult, ALU.max)
                    pb = p_pool.tile([128, JGRP * RPC], BF16, tag="pb")
                    nc.scalar.activation(pb[:], wb[:], AFT.Exp)
                    for j8 in range(JGRP):
                        jc = g * JGRP + j8
                        rhs = hs[:, blk(jc):blk(jc) + RH]
                        for s in range(NIS):
                            nc.tensor.matmul(
                                aps[s][:, :RH],
                                pb[:, j8 * RPC + s * 128:
                                   j8 * RPC + (s + 1) * 128],
                                rhs,
                                start=(jc == 0), stop=(jc == NJC - 1))
                # epilogue
                for s in range(NIS):
                    zr = eps_pool.tile([128, 1], F32, tag="zr")
                    nc.vector.reciprocal(zr[:], aps[s][:, hid:hid + 1])
                    if label == "l1":
                        tt = eps_pool.tile([128, hid], BF16, tag="tt")
                        nc.scalar.activation(tt[:], aps[s][:, :hid], AFT.Copy,
                                             bias=0.0, scale=zr[:])
                        tm = eps_pool.tile([128, hid], BF16, tag="tm")
                        nc.vector.tensor_scalar(tm[:], tt[:], 0.0, None,
                                                ALU.min)
                        te = eps_pool.tile([128, hid], BF16, tag="te")
                        nc.scalar.activation(te[:], tm[:], AFT.Exp)
                        tr = eps_pool.tile([128, hid], BF16, tag="tr")
                        nc.vector.tensor_scalar(tr[:], tt[:], 0.0, -1.0,
                                                ALU.max, ALU.add)
                        nc.vector.tensor_add(
                            xcts[s][:, o * hid:(o + 1) * hid], te[:], tr[:])
                        if o == nheads - 1:
                            nc.sync.dma_start(
                                out_d.ap()[b, s * 128:(s + 1) * 128, :],
                                xcts[s][:])
                    else:
                        ot = out_pool.tile([128, hid], F32, tag="ot")
                        nc.scalar.activation(ot[:], aps[s][:, :hid], AFT.Copy,
                                             bias=0.0, scale=zr[:])
                        nc.sync.dma_start(
                            out_d.ap()[b, s * 128:(s + 1) * 128, :], ot[:])

    nc.compile()
    return nc


def _get_programs():
    if "l1" not in _cache:
        _cache["l1"] = _build_layer(H, HID, "l1")
    if "l2" not in _cache:
        _cache["l2"] = _build_layer(1, OUT, "l2")
    return _cache["l1"], _cache["l2"]


def _aug_weights(W, a1, a2, nheads, hid):
    W = W.reshape(nheads, hid, F_IN).astype(np.float32)
    a1 = a1.reshape(nheads, hid).astype(np.float32)
    a2 = a2.reshape(nheads, hid).astype(np.float32)
    w1 = np.einsum("ohf,oh->of", W, a1)   # [o, F_IN]
    w2 = np.einsum("ohf,oh->of", W, a2)
    waug = np.zeros((nheads, NKC, 128, hid), np.float32)
    w1c = np.zeros((nheads, NKC, 128, 1), np.float32)
    w2c = np.zeros((nheads, NKC, 128, 1), np.float32)
    for o in range(nheads):
        WT = W[o].T  # [F_IN, hid]
        for kc in range(NKC):
            sl = slice(kc * 128, (kc + 1) * 128)
            waug[o, kc] = WT[sl]
            w1c[o, kc, :, 0] = w1[o, sl]
            w2c[o, kc, :, 0] = w2[o, sl]
    return (waug.astype(NPBF16), w1c.astype(NPBF16), w2c.astype(NPBF16))


def _run(nc, in_maps):
    trace = bool(int(os.environ.get("GAT_TRACE", "0")))
    res = run_bass_kernel_spmd(nc, in_maps, list(range(NCORES)), trace=trace)
    if res.exec_time_ns is not None:
        last_exec_ns.append(res.exec_time_ns)
    return res


def kernel(**inputs):
    global last_exec_ns
    last_exec_ns = []
    x = np.asarray(inputs["x"], np.float32)
    adj = np.asarray(inputs["adj"])
    W_heads = np.asarray(inputs["W_heads"], np.float32)
    a1_heads = np.asarray(inputs["a1_heads"], np.float32)
    a2_heads = np.asarray(inputs["a2_heads"], np.float32)
    W_out = np.asarray(inputs["W_out"], np.float32)
    a1_out = np.asarray(inputs["a1_out"], np.float32)
    a2_out = np.asarray(inputs["a2_out"], np.float32)

    nc1, nc2 = _get_programs()

    xT = np.ascontiguousarray(x.transpose(0, 2, 1)).astype(NPBF16)  # [B,F,N]
    waug1, w11, w21 = _aug_weights(W_heads, a1_heads, a2_heads, H, HID)
    waug2, w12, w22 = _aug_weights(W_out[None], a1_out[None], a2_out[None],
                                   1, OUT)

    masks = []
    for c in range(NCORES):
        sl = slice(c * RPC, (c + 1) * RPC)
        m = (adj[:, sl, :].transpose(0, 2, 1).astype(np.float32) - 1.0) * MASKVAL
        masks.append(np.ascontiguousarray(m).astype(NPBF16))

    in_maps1 = []
    for c in range(NCORES):
        sl = slice(c * RPC, (c + 1) * RPC)
        in_maps1.append({
            "xT": xT,
            "xT_own": np.ascontiguousarray(xT[:, :, sl]),
            "maskT": masks[c],
            "w_aug": waug1,
            "w1": w11,
            "w2": w21,
        })
    r1 = _run(nc1, in_maps1)
    xcat = np.concatenate(
        [r1.results[c]["xcat"].astype(np.float32) for c in range(NCORES)],
        axis=1)  # [B, N, H*HID]
    xcatT = np.ascontiguousarray(xcat.transpose(0, 2, 1)).astype(NPBF16)

    in_maps2 = []
    for c in range(NCORES):
        sl = slice(c * RPC, (c + 1) * RPC)
        in_maps2.append({
            "xT": xcatT,
            "xT_own": np.ascontiguousarray(xcatT[:, :, sl]),
            "maskT": masks[c],
            "w_aug": waug2,
            "w1": w12,
            "w2": w22,
        })
    r2 = _run(nc2, in_maps2)
    out = np.concatenate(
        [r2.results[c]["out"] for c in range(NCORES)], axis=1)
    return out.astype(np.float32)


# revision 17
# speedup vs baseline: 19737.6456x; 19737.6456x over previous
"""GAT (2-layer graph attention) Trainium2 kernel, 8-core SPMD.

Sharding: rows of N are sharded across 8 cores (512 rows each); every core
computes the full first-layer projections h (replicated) but only its own
rows' attention. Between layers the per-core x_cat rows are gathered on the
host and layer 2 runs as a second launch.

Math per layer (per batch b, head o):
    h = x @ W^T, f1 = x (W^T a1), f2 = x (W^T a2)
    P^T[j,i] = exp(lrelu(f1_i + f2_j + M[j,i]))   M = (adj^T-1)*3e4 (mask)
    [U | Z] = P^T.T @ [h | 1]  (PE accumulates over j-chunks)
    out_i = U_i / Z_i  (+ ELU for layer 1)

Layout: scores kept transposed [j(part), i(free)] so the PE contraction dim
(j) is on partitions for both operands; Z comes free via the ones column.
f2 is produced partition-major by tiny N=PAIR matmuls that reuse the already
loaded x^T weights; f1 (free-major) by M=1 matmuls + a ones outer-product.
"""

import os
from contextlib import ExitStack

import numpy as np
import ml_dtypes

import concourse.bacc as bacc
import concourse.tile as tile
import concourse.mybir as mybir
from concourse.bass_utils import run_bass_kernel_spmd

BF16 = mybir.dt.bfloat16
F32 = mybir.dt.float32
NPBF16 = ml_dtypes.bfloat16
AFT = mybir.ActivationFunctionType
ALU = mybir.AluOpType

NCORES = 8
B, N, F_IN, H, HID, OUT = 2, 4096, 512, 4, 128, 64
RPC = N // NCORES          # 512 rows per core
NJC = N // 128             # 32 column (j) chunks
NKC = F_IN // 128          # 4 contraction chunks for projections
NIS = RPC // 128           # 4 row (i) subtiles per core
MASKVAL = 30000.0
JGRP = 4                   # j-chunks per batched exp group
NGRP = NJC // JGRP
MSG = 8                    # ms per xT slab load

# engine-assignment config (tuned against TimelineSim)
CFG = {
    "tt_dve_mod": 4,        # every k-th u-build TT stays on DVE (rest gpsimd)
}

_cache = {}

last_exec_ns = []


def _build_layer(nheads, hid, label):
    nc = bacc.Bacc("TRN2", target_bir_lowering=False, debug=False,
                   enable_asserts=True, num_devices=NCORES)

    PAIR = 2 if nheads % 2 == 0 else 1
    NPR = nheads // PAIR   # head pairs
    HS = hid + 4           # hsb per-head block stride: [h | 1 | pad]
    RH = hid + 1           # attn rhs width: [h | 1]

    xT = nc.dram_tensor("xT", [B, F_IN, N], BF16, kind="ExternalInput")
    xT_own = nc.dram_tensor("xT_own", [B, F_IN, RPC], BF16, kind="ExternalInput")
    maskT = nc.dram_tensor("maskT", [B, N, RPC], BF16, kind="ExternalInput")
    NPR_ = nheads // (2 if nheads % 2 == 0 else 1)
    PAIR_ = nheads // NPR_
    w_aug = nc.dram_tensor("w_aug", [128, nheads * NKC * hid], BF16,
                           kind="ExternalInput")
    w1 = nc.dram_tensor("w1", [128, nheads * NKC], BF16, kind="ExternalInput")
    w2 = nc.dram_tensor("w2", [128, nheads * NKC], BF16, kind="ExternalInput")
    if label == "l1":
        out_d = nc.dram_tensor("xcat", [B, RPC, nheads * hid], BF16,
                               kind="ExternalOutput")
    else:
        out_d = nc.dram_tensor("out", [B, RPC, hid], F32, kind="ExternalOutput")

    with tile.TileContext(nc) as tc, ExitStack() as ctx:
        const_pool = ctx.enter_context(tc.tile_pool(name="const", bufs=1))
        mask_pool = ctx.enter_context(tc.tile_pool(name="mask", bufs=2))
        xt_pool = ctx.enter_context(tc.tile_pool(name="xt", bufs=2 * NKC))
        xto_pool = ctx.enter_context(tc.tile_pool(name="xto", bufs=2 * NKC))
        hsb_pool = ctx.enter_context(tc.tile_pool(name="hsb", bufs=NPR + 1))
        f2s_pool = ctx.enter_context(tc.tile_pool(name="f2s", bufs=NPR + 1))
        f1b_pool = ctx.enter_context(tc.tile_pool(name="f1b", bufs=nheads + 1))
        f1r_pool = ctx.enter_context(tc.tile_pool(name="f1r", bufs=2))
        u_pool = ctx.enter_context(tc.tile_pool(name="u", bufs=3))
        w_pool = ctx.enter_context(tc.tile_pool(name="wb", bufs=3))
        p_pool = ctx.enter_context(tc.tile_pool(name="pb", bufs=3))
        eps_pool = ctx.enter_context(tc.tile_pool(name="eps", bufs=2))
        out_pool = ctx.enter_context(tc.tile_pool(name="outp", bufs=2 * NIS))
        hpsum = ctx.enter_context(tc.tile_pool(name="hpsum", bufs=4, space="PSUM"))
        apsum = ctx.enter_context(tc.tile_pool(name="apsum", bufs=4, space="PSUM"))

        # constants: waug_sb holds [W^T_a | W^T_b] per (pair, kc); w2_sb holds
        # [w2_a | w2_b] per (pair, kc); w1_sb one col per (head, kc)
        waug_sb = const_pool.tile([128, NPR * NKC * PAIR * hid], BF16)
        w1_sb = const_pool.tile([128, nheads * NKC], BF16)
        w2_sb = const_pool.tile([128, NPR * NKC * PAIR], BF16)
        # host pre-bakes the SBUF layouts: one contiguous DMA each
        nc.sync.dma_start(waug_sb[:], w_aug.ap()[:, :])
        nc.sync.dma_start(w1_sb[:], w1.ap()[:, :])
        nc.sync.dma_start(w2_sb[:], w2.ap()[:, :])
        ones_sb = const_pool.tile([1, 128], BF16)
        nc.vector.memset(ones_sb[:], 1.0)

        for b in range(B):
            masksb = mask_pool.tile([128, NJC * RPC], BF16, tag="masksb")
            NQ = NJC // 4

            def load_mask_quarter(mq):
                nc.sync.dma_start(
                    masksb[:, mq * NQ * RPC:(mq + 1) * NQ * RPC].rearrange(
                        "p (c i) -> p c i", i=RPC),
                    maskT.ap()[b, mq * NQ * 128:(mq + 1) * NQ * 128].rearrange(
                        "(c p) i -> p c i", p=128))

            load_mask_quarter(0)

            # --- f1 (own rows, free-major): M=1 matmuls + ones outer-product
            xto = []
            for kc in range(NKC):
                t = xto_pool.tile([128, RPC], BF16, tag="xto")
                nc.sync.dma_start(t[:], xT_own.ap()[b, kc * 128:(kc + 1) * 128, :])
                xto.append(t)
            f1bs = []
            for o in range(nheads):
                fps = hpsum.tile([128, 512], F32, tag="hps", name="fps")
                for kc in range(NKC):
                    nc.tensor.matmul(fps[0:1, :RPC],
                                     w1_sb[:, o * NKC + kc:o * NKC + kc + 1],
                                     xto[kc][:, :],
                                     start=(kc == 0), stop=(kc == NKC - 1))
                f1r = f1r_pool.tile([1, RPC], BF16, tag="f1r")
                nc.vector.tensor_copy(f1r[:], fps[0:1, :RPC])
                bps = hpsum.tile([128, 512], F32, tag="hps")
                nc.tensor.matmul(bps[:, :RPC], ones_sb[:, :], f1r[:, :],
                                 start=True, stop=True)
                f1b = f1b_pool.tile([128, RPC], BF16, tag="f1b")
                nc.scalar.copy(f1b[:], bps[:, :RPC])
                f1bs.append(f1b)

            # --- projections: h (pairs) + f2 (partition-major) ---
            hsbs = []    # per pair: [128, NJC*PAIR*HS]
            for pr in range(NPR):
                hs = hsb_pool.tile([128, NJC * PAIR * HS], BF16, tag="hs")
                hv = hs[:].rearrange("p (c q f) -> p c q f", q=PAIR, f=HS)
                nc.vector.memset(hv[:, :, :, hid:hid + 1], 1.0)
                hsbs.append(hs)
            f2ps = [hpsum.tile([128, 512], F32, tag="hps", name="f2ps")
                    for _ in range(NPR)]
            f2sbs = [f2s_pool.tile([128, NJC * PAIR], F32, tag="f2sb",
                                   name="f2sb")
                     for _ in range(NPR)]
            for ms in range(NJC):
                if ms == MSG:
                    for _mq in range(1, 4):
                        load_mask_quarter(_mq)
                if ms % MSG == 0:
                    xslabs = []
                    for kc in range(NKC):
                        xs = xt_pool.tile([128, MSG * 128], BF16, tag="xt")
                        nc.sync.dma_start(
                            xs[:],
                            xT.ap()[b, kc * 128:(kc + 1) * 128,
                                    ms * 128:(ms + MSG) * 128])
                        xslabs.append(xs)
                hps = [hpsum.tile([128, 512], F32, tag="hps", name="hps")
                       for _ in range(NPR)]
                for kc in range(NKC):
                    xt_t = xslabs[kc][:, (ms % MSG) * 128:(ms % MSG + 1) * 128]
                    st = (kc == 0)
                    sp = (kc == NKC - 1)
                    for pr in range(NPR):
                        i = (pr * NKC + kc) * PAIR
                        nc.tensor.matmul(
                            hps[pr][:, :PAIR * hid], xt_t,
                            waug_sb[:, i * hid:(i + PAIR) * hid],
                            start=st, stop=sp)
                        nc.tensor.matmul(
                            f2ps[pr][:, ms * PAIR:(ms + 1) * PAIR], xt_t,
                            w2_sb[:, i:i + PAIR],
                            start=st, stop=sp)
                for pr in range(NPR):
                    hv = hsbs[pr][:].rearrange("p (c q f) -> p c q f",
                                               q=PAIR, f=HS)
                    pv = hps[pr][:, :PAIR * hid].rearrange(
                        "p (q f) -> p q f", f=hid)
                    nc.scalar.copy(hv[:, ms, :, 0:hid], pv)
                # f2 to fp32 SBUF incrementally so attention can pipeline
                if ms % JGRP == JGRP - 1:
                    lo = (ms - (JGRP - 1)) * PAIR
                    hi = (ms + 1) * PAIR
                    for pr in range(NPR):
                        nc.vector.tensor_copy(f2sbs[pr][:, lo:hi],
                                              f2ps[pr][:, lo:hi])

            # --- attention per head ---
            if label == "l1":
                xcts = [out_pool.tile([128, nheads * hid], BF16, tag="xct",
                                      name="xct")
                        for _ in range(NIS)]
            for o in range(nheads):
                pr, q = o // PAIR, o % PAIR
                hs = hsbs[pr]
                f2sb = f2sbs[pr]

                def blk(jc):
                    return (jc * PAIR + q) * HS

                aps = [apsum.tile([128, 512], F32, tag="aps", name="aps")
                       for _ in range(NIS)]
                for g in range(NGRP):
                    wb = w_pool.tile([128, JGRP * RPC], BF16, tag="wb")
                    ub = u_pool.tile([128, JGRP * RPC], BF16, tag="ub")
                    for j8 in range(JGRP):
                        jc = g * JGRP + j8
                        usl = ub[:, j8 * RPC:(j8 + 1) * RPC]
                        gidx = ((b * nheads + o) * NGRP + g) * JGRP + j8
                        m = CFG["tt_dve_mod"]
                        tte = nc.vector if (m and gidx % m == 0) else nc.gpsimd
                        tte.tensor_add(usl, masksb[:, jc * RPC:(jc + 1) * RPC],
                                       f1bs[o][:])
                        nc.vector.tensor_scalar(
                            usl, usl, f2sb[:, jc * PAIR + q:jc * PAIR + q + 1],
                            None, ALU.add)
                    # lrelu: one batched STT on DVE per group
                    nc.vector.scalar_tensor_tensor(
                        wb[:], ub[:], 0.2, ub[:], ALU.mult, ALU.max)
                    pb = p_pool.tile([128, JGRP * RPC], BF16, tag="pb")
                    nc.scalar.activation(pb[:], wb[:], AFT.Exp)
                    for j8 in range(JGRP):
                        jc = g * JGRP + j8
                        rhs = hs[:, blk(jc):blk(jc) + RH]
                        for s in range(NIS):
                            nc.tensor.matmul(
                                aps[s][:, :RH],
                                pb[:, j8 * RPC + s * 128:
                                   j8 * RPC + (s + 1) * 128],
                                rhs,
                                start=(jc == 0), stop=(jc == NJC - 1))
                # epilogue
                for s in range(NIS):
                    zr = eps_pool.tile([128, 1], F32, tag="zr")
                    nc.vector.reciprocal(zr[:], aps[s][:, hid:hid + 1])
                    if label == "l1":
                        tt = eps_pool.tile([128, hid], BF16, tag="tt")
                        nc.scalar.activation(tt[:], aps[s][:, :hid], AFT.Copy,
                                             bias=0.0, scale=zr[:])
                        tm = eps_pool.tile([128, hid], BF16, tag="tm")
                        nc.vector.tensor_scalar(tm[:], tt[:], 0.0, None,
                                                ALU.min)
                        te = eps_pool.tile([128, hid], BF16, tag="te")
                        nc.scalar.activation(te[:], tm[:], AFT.Exp)
                        tr = eps_pool.tile([128, hid], BF16, tag="tr")
                        nc.vector.tensor_scalar(tr[:], tt[:], 0.0, -1.0,
                                                ALU.max, ALU.add)
                        nc.vector.tensor_add(
                            xcts[s][:, o * hid:(o + 1) * hid], te[:], tr[:])
                        if o == nheads - 1:
                            nc.sync.dma_start(
                                out_d.ap()[b, s * 128:(s + 1) * 128, :],
                                xcts[s][:])
                    else:
                        ot = out_pool.tile([128, hid], F32, tag="ot")
                        nc.scalar.activation(ot[:], aps[s][:, :hid], AFT.Copy,
                                             bias=0.0, scale=zr[:])
                        nc.sync.dma_start(
                            out_d.ap()[b, s * 128:(s + 1) * 128, :], ot[:])

    nc.compile()
    return nc


def _get_programs():
    if "l1" not in _cache:
        _cache["l1"] = _build_layer(H, HID, "l1")
    if "l2" not in _cache:
        _cache["l2"] = _build_layer(1, OUT, "l2")
    return _cache["l1"], _cache["l2"]


def _aug_weights(W, a1, a2, nheads, hid):
    """Bake W^T / w1 / w2 into the exact SBUF layouts the kernel loads."""
    W = W.reshape(nheads, hid, F_IN).astype(np.float32)
    a1 = a1.reshape(nheads, hid).astype(np.float32)
    a2 = a2.reshape(nheads, hid).astype(np.float32)
    w1 = np.einsum("ohf,oh->of", W, a1)   # [o, F_IN]
    w2 = np.einsum("ohf,oh->of", W, a2)
    PAIR = 2 if nheads % 2 == 0 else 1
    NPR = nheads // PAIR
    waug = np.zeros((128, nheads * NKC * hid), np.float32)
    w1c = np.zeros((128, nheads * NKC), np.float32)
    w2c = np.zeros((128, nheads * NKC), np.float32)
    for o in range(nheads):
        pr, q = o // PAIR, o % PAIR
        WT = W[o].T  # [F_IN, hid]
        for kc in range(NKC):
            sl = slice(kc * 128, (kc + 1) * 128)
            i = (pr * NKC + kc) * PAIR + q
            waug[:, i * hid:(i + 1) * hid] = WT[sl]
            w1c[:, o * NKC + kc] = w1[o, sl]
            w2c[:, i] = w2[o, sl]
    return (waug.astype(NPBF16), w1c.astype(NPBF16), w2c.astype(NPBF16))


def _run(nc, in_maps):
    trace = bool(int(os.environ.get("GAT_TRACE", "0")))
    res = run_bass_kernel_spmd(nc, in_maps, list(range(NCORES)), trace=trace)
    if res.exec_time_ns is not None:
        last_exec_ns.append(res.exec_time_ns)
    return res


def kernel(**inputs):
    global last_exec_ns
    last_exec_ns = []
    x = np.asarray(inputs["x"], np.float32)
    adj = np.asarray(inputs["adj"])
    W_heads = np.asarray(inputs["W_heads"], np.float32)
    a1_heads = np.asarray(inputs["a1_heads"], np.float32)
    a2_heads = np.asarray(inputs["a2_heads"], np.float32)
    W_out = np.asarray(inputs["W_out"], np.float32)
    a1_out = np.asarray(inputs["a1_out"], np.float32)
    a2_out = np.asarray(inputs["a2_out"], np.float32)

    nc1, nc2 = _get_programs()

    xT = np.ascontiguousarray(x.transpose(0, 2, 1)).astype(NPBF16)  # [B,F,N]
    waug1, w11, w21 = _aug_weights(W_heads, a1_heads, a2_heads, H, HID)
    waug2, w12, w22 = _aug_weights(W_out[None], a1_out[None], a2_out[None],
                                   1, OUT)

    masks = []
    for c in range(NCORES):
        sl = slice(c * RPC, (c + 1) * RPC)
        m = (adj[:, sl, :].transpose(0, 2, 1).astype(np.float32) - 1.0) * MASKVAL
        masks.append(np.ascontiguousarray(m).astype(NPBF16))

    in_maps1 = []
    for c in range(NCORES):
        sl = slice(c * RPC, (c + 1) * RPC)
        in_maps1.append({
            "xT": xT,
            "xT_own": np.ascontiguousarray(xT[:, :, sl]),
            "maskT": masks[c],
            "w_aug": waug1,
            "w1": w11,
            "w2": w21,
        })
    r1 = _run(nc1, in_maps1)
    xcat = np.concatenate(
        [r1.results[c]["xcat"].astype(np.float32) for c in range(NCORES)],
        axis=1)  # [B, N, H*HID]
    xcatT = np.ascontiguousarray(xcat.transpose(0, 2, 1)).astype(NPBF16)

    in_maps2 = []
    for c in range(NCORES):
        sl = slice(c * RPC, (c + 1) * RPC)
        in_maps2.append({
            "xT": xcatT,
            "xT_own": np.ascontiguousarray(xcatT[:, :, sl]),
            "maskT": masks[c],
            "w_aug": waug2,
            "w1": w12,
            "w2": w22,
        })
    r2 = _run(nc2, in_maps2)
    out = np.concatenate(
        [r2.results[c]["out"] for c in range(NCORES)], axis=1)
    return out.astype(np.float32)
